# revision 35
# baseline (speedup 1.0000x reference)
"""CIN (Compressed Interaction Network) forward kernel for Trainium2.

Hybrid square-trick / broadcast formulation. The CIN layers contract
z[(a,b), r] = v_a[r]*w_b[r] against weights. Two mechanisms produce the
z chunks, chosen per chunk to balance the ACT and DVE engines:

* square trick (ACT): a*b = 1/2[(a+b)^2 - a^2 - b^2]. The PE selection
  matmul produces the SUMS a+b directly (two 1s per sel column) as an fp8
  hi/lo DoubleRow matmul at 0.5 cycles/row; ACT's Square op then turns the
  PSUM sums into z products in a single pass (an instruction may read only
  ONE input from PSUM, so the squares cannot run as DVE mult(s,s)). The
  -a^2/-b^2 terms fold into small correction matmuls (x^2 and cur0^2
  computed once per r-tile) whose host-precomputed weights sum only over
  square-trick rows.
* direct product (DVE): fp8 hi/lo DoubleRow broadcast of x_h (single 1
  per sel column) + one DVE multiply bc (x) cur0 with the PSUM tile as the
  single PSUM operand.

Layer 0 uses the square trick only (it symmetrizes: x_h*x_m needs just
h<m pairs, 496 rows + 32 x^2-correction rows vs 1024, so 4 chunks not 8).
Layer 1 splits its 32 h-rows: H_SQ h's use square-trick chunks packed in
V-tile m-blocks, the rest use direct chunks (fixed h, m = partition).
The square-trick rhs are V-tiles stacking x (static fp8 hi/lo) with relu0
output quantized to fp8 hi/lo on the fly (Pool copy + subtract). Layer-0
output channels are rotated by 32 partitions so every engine op stays
partition-aligned (HW: engine APs starting at partition 32 span <=32
partitions); host-side permutations of W0'/corr columns, b1/W1-direct
rows and y0 rows undo it.

Per r-tile engine load (TimelineSim): PE 36 bcasts x 106.7 + 39 mains x
213 = 12.2 us (89% busy); ACT (2 L0 + 8 L1) pair-squares + 2 relu = 11.6
us; DVE 8 pair-multiplies + 2 reduces = 10.2 us; Pool quantize + x^2 +
cur0^2 = 8.2 us. 32 r-tiles -> 438 us/core vs 585 us for the pure
broadcast+multiply formulation (rel err 4.1e-3 vs 3.4e-3).
"""

import os
import sys

if "/opt/trn_rl_repo" not in sys.path:
    sys.path.insert(0, "/opt/trn_rl_repo")

import ml_dtypes
import numpy as np

import concourse.bacc as bacc
import concourse.bass as bass
import concourse.mybir as mybir
from concourse.bass_utils import run_bass_kernel_spmd
from concourse.bass_types import AP
from concourse.tile import TileContext

f32 = mybir.dt.float32
bf16 = mybir.dt.bfloat16
f16 = mybir.dt.float16
f8 = mybir.dt.float8e4
np_bf16 = ml_dtypes.bfloat16
np_f8 = ml_dtypes.float8_e4m3

NCORES = 8
B, F, D = 4096, 32, 32
N0 = N1 = 128
BC = B // NCORES            # 512 batches per core
R = BC * D                  # 16384 (b, d) columns per core
RT = 512                    # free-dim tile (one fp32 PSUM bank)
NRT = R // RT               # 32
ROT = 32                    # layer-0 channel rotation (partition alignment)

# layer-0 symmetric pairs h<m, lexicographic
PAIRS0 = [(h, m) for h in range(F) for m in range(h + 1, F)]      # 496
L0C = 4                                                           # chunks

# layer-1 mechanism split: h < hsq -> square trick (ACT), else direct (DVE).
# The split alternates per r-tile (SPLITPAT[rt % len]) to average the ACT
# and DVE loads below the PE's; each hsq must be a multiple of 4.
SPLITPAT = [int(x) for x in os.environ.get("CIN_SPLITPAT", "16").split(",")]
VARIANTS = sorted(set(SPLITPAT))


def _l1_blks(hsq):
    h_dir = list(range(hsq, F))
    blk1 = [(h, m) for h in range(hsq) for m in range(96)]
    blk2 = [(h, m) for h in range(hsq) for m in range(96, 128)]
    return h_dir, blk1, blk2

LOOKAHEAD = int(os.environ.get("CIN_LOOKAHEAD", "2"))   # pair-units ahead
LAG = int(os.environ.get("CIN_LAG", "1"))               # L1 runs after L0(rt+LAG)
BUFS_ZT = int(os.environ.get("CIN_BUFS_ZT", "6"))
REPEAT = int(os.environ.get("CIN_REPEAT", "1"))


def _dup_halves(sel):
    """[C, K, 128] -> [K, C*128] fp8; the DoubleRow lhsT reads each chunk
    twice via a stride-0 half-dimension (half 0 hits hi, half 1 lo)."""
    c, k, p = sel.shape
    return np.ascontiguousarray(sel.transpose(1, 0, 2)).reshape(k, c * 128).astype(np_f8)


def _sel0_const():
    s = np.zeros((L0C, 32, 128), dtype=np.float32)
    for i, (h, m) in enumerate(PAIRS0):
        c, p = divmod(i, 128)
        s[c, h, p] = 1.0
        s[c, m, p] = 1.0
    return _dup_halves(s)


def _sel1a_const(hsq):
    _, blk1, _ = _l1_blks(hsq)
    s = np.zeros((len(blk1) // 128, 128, 128), dtype=np.float32)
    for i, (h, m) in enumerate(blk1):
        c, p = divmod(i, 128)
        s[c, h, p] = 1.0           # x row
        s[c, 32 + m, p] = 1.0      # cur0 channel m (V1 row 32+m)
    return _dup_halves(s)


def _sel1b_const(hsq):
    _, _, blk2 = _l1_blks(hsq)
    s = np.zeros((len(blk2) // 128, 64, 128), dtype=np.float32)
    for i, (h, m) in enumerate(blk2):
        c, p = divmod(i, 128)
        s[c, m - 96, p] = 1.0      # cur0 channel m (V2 row m-96)
        s[c, 32 + h, p] = 1.0      # x row (V2 row 32+h)
    return _dup_halves(s)


def _sel1d_const(hsq):
    """Direct-mechanism broadcast: chunk per h in h_dir, x_h on all 128."""
    h_dir, _, _ = _l1_blks(hsq)
    s = np.zeros((len(h_dir), 32, 128), dtype=np.float32)
    for c, h in enumerate(h_dir):
        s[c, h, :] = 1.0
    return _dup_halves(s)


def build_nc() -> bass.Bass:
    nc = bacc.Bacc("TRN2", name="cin_sq")
    x8d = nc.dram_tensor("x8", [F, 2 * R], f8, kind="ExternalInput")
    xtd = nc.dram_tensor("xt", [F, R], bf16, kind="ExternalInput")
    w0pd = nc.dram_tensor("w0p", [128, L0C * 128], f16, kind="ExternalInput")
    nv = len(VARIANTS)
    vd = {}
    for v in VARIANTS:
        vd[v] = {
            "w1p": nc.dram_tensor(f"w1p{v}", [128, v * 128], f16,
                                  kind="ExternalInput"),
            "w1d": nc.dram_tensor(f"w1d{v}", [128, (F - v) * 128], f16,
                                  kind="ExternalInput"),
            "sel1a": nc.inline_tensor(_sel1a_const(v), name=f"sel1a{v}"),
            "sel1b": nc.inline_tensor(_sel1b_const(v), name=f"sel1b{v}"),
            "sel1d": nc.inline_tensor(_sel1d_const(v), name=f"sel1d{v}"),
        }
    # wcb: per-variant b1 [128x128], shared corr0 [32x128], per-variant a1
    wcbd = nc.dram_tensor("wcb", [128, 128 * (2 * nv + 1)], f16,
                          kind="ExternalInput")
    y = nc.dram_tensor("y", [2, 128, BC], f16, kind="ExternalOutput")
    sel0_d = nc.inline_tensor(_sel0_const(), name="sel0")

    with TileContext(nc) as tc:
        with (
            tc.tile_pool(name="singles", bufs=1) as singles,
            tc.tile_pool(name="zt", bufs=BUFS_ZT) as zt_pool,
            tc.tile_pool(name="t", bufs=2 + max(1, LAG)) as t_pool,
            tc.tile_pool(name="x2", bufs=4 + LAG) as x2_pool,
            tc.tile_pool(name="c2", bufs=2 + max(1, LAG)) as c2_pool,
            tc.tile_pool(name="relu1", bufs=2) as relu1_pool,
            tc.tile_pool(name="psum_bc", bufs=3, space="PSUM") as psum_bc,
            tc.tile_pool(name="psum_a0", bufs=1, space="PSUM") as psum_a0,
            tc.tile_pool(name="psum_a1", bufs=1, space="PSUM") as psum_a1,
        ):
            # --- resident tensors --------------------------------------
            V1 = singles.tile([128, 2 * R], f8)   # 0:32 x hi|lo, 32:128 cur0 ch 0:96
            V2 = singles.tile([64, 2 * R], f8)    # 0:32 cur0 ch 96:128, 32:64 x
            xt_sb = singles.tile([32, R], bf16)
            sel0 = singles.tile([32, L0C * 128], f8)
            w0p = singles.tile([128, L0C * 128], f16)
            vt = {}
            for v in VARIANTS:
                na, nb = v * 96 // 128, v * 32 // 128
                vt[v] = {
                    "sel1a": singles.tile([128, na * 128], f8, name=f"sel1a{v}"),
                    "sel1b": singles.tile([64, nb * 128], f8, name=f"sel1b{v}"),
                    "sel1d": singles.tile([32, (F - v) * 128], f8, name=f"sel1d{v}"),
                    "w1p": singles.tile([128, v * 128], f16, name=f"w1p{v}"),
                    "w1d": singles.tile([128, (F - v) * 128], f16, name=f"w1d{v}"),
                }
            wcb = singles.tile([128, 128 * (2 * nv + 1)], f16)
            for vi, v in enumerate(VARIANTS):
                vt[v]["b1"] = wcb[:, 128 * vi:128 * (vi + 1)]
                vt[v]["a1"] = wcb[0:32, 128 * (nv + 1 + vi):128 * (nv + 2 + vi)]
            wc0 = wcb[0:32, 128 * nv:128 * (nv + 1)]
            # fp16 reduce outputs: 2-byte packed APs let the DVE reduces
            # run in 2x mode; the d-sum of 32 relu terms loses <5e-4 rel
            y0 = singles.tile([128, BC], f16)
            y1 = singles.tile([128, BC], f16)

            # --- input DMAs, ordered by first-use time -----------------
            # Transfers serialize on the DMA device early on, so order is:
            # minimal head to start compute, then weights/sels by first use,
            # then the big fp8 x planes streamed in fine splits.
            head = min(int(os.environ.get("CIN_XHEAD", "1024")), R)

            def hilo_cols(tile, prange, cs):
                a = tile[prange[0]:prange[1], cs]
                return AP(a.tensor, a.offset,
                          [a.ap[0], [R, 2], [1, cs.stop - cs.start]])

            def x8_src(cs):
                a = x8d[:, cs]
                return AP(a.tensor, a.offset,
                          [a.ap[0], [R, 2], [1, cs.stop - cs.start]])

            nc.sync.dma_start(out=sel0[:, :], in_=sel0_d[:, :])
            nc.scalar.dma_start(out=w0p[:, :], in_=w0pd[:, :])
            nc.sync.dma_start(out=hilo_cols(V1, (0, 32), slice(0, head)),
                              in_=x8_src(slice(0, head)))
            nc.scalar.dma_start(out=xt_sb[:, 0:head], in_=xtd[:, 0:head])
            nc.gpsimd.dma_start(out=wcb[:, :], in_=wcbd[:, :])
            v0 = SPLITPAT[0]
            nc.sync.dma_start(out=vt[v0]["sel1d"][:, :], in_=vd[v0]["sel1d"][:, :])
            nc.scalar.dma_start(out=vt[v0]["w1d"][:, :], in_=vd[v0]["w1d"][:, :])
            nc.sync.dma_start(out=vt[v0]["sel1a"][:, :], in_=vd[v0]["sel1a"][:, :])
            nc.scalar.dma_start(out=vt[v0]["w1p"][:, :], in_=vd[v0]["w1p"][:, :])
            nc.sync.dma_start(out=vt[v0]["sel1b"][:, :], in_=vd[v0]["sel1b"][:, :])
            nc.scalar.dma_start(out=hilo_cols(V2, (32, 64), slice(0, head)),
                              in_=x8_src(slice(0, head)))
            for v in VARIANTS:
                if v == v0:
                    continue
                nc.gpsimd.dma_start(out=vt[v]["sel1d"][:, :], in_=vd[v]["sel1d"][:, :])
                nc.sync.dma_start(out=vt[v]["w1d"][:, :], in_=vd[v]["w1d"][:, :])
                nc.scalar.dma_start(out=vt[v]["sel1a"][:, :], in_=vd[v]["sel1a"][:, :])
                nc.gpsimd.dma_start(out=vt[v]["w1p"][:, :], in_=vd[v]["w1p"][:, :])
                nc.sync.dma_start(out=vt[v]["sel1b"][:, :], in_=vd[v]["sel1b"][:, :])
            rest = R - head
            nsplit = 2
            dma_engines = [nc.sync, nc.scalar, nc.gpsimd]
            for s in range(nsplit if rest > 0 else 0):
                cs = slice(head + s * (rest // nsplit),
                           head + (s + 1) * (rest // nsplit))
                dma_engines[s % 3].dma_start(
                    out=hilo_cols(V1, (0, 32), cs), in_=x8_src(cs))
                dma_engines[(s + 1) % 3].dma_start(
                    out=hilo_cols(V2, (32, 64), cs), in_=x8_src(cs))
                dma_engines[(s + 2) % 3].dma_start(
                    out=xt_sb[:, cs], in_=xtd[:, cs])

            # --- helpers -----------------------------------------------
            def hilo_ap(tile, k, rs):
                a = tile[0:k, rs]
                return AP(a.tensor, a.offset,
                          [a.ap[0], [R, 2], [1, rs.stop - rs.start]])

            def sel_ap(tile, ci):
                s = tile[:, ci * 128:(ci + 1) * 128]
                return AP(s.tensor, s.offset, [s.ap[0], [0, 2], [1, 128]])

            def pair_ap(t):
                """[128, RT] -> logical [128, 2, RT] via a step-0 dim."""
                a = t if isinstance(t, AP) else t[:, :]
                return AP(a.tensor, a.offset, [a.ap[0], [0, 2], a.ap[1]])

            def dr_sum(sel_tile, ci, rhs_tile, k, rs, out_ap):
                """fp8 DoubleRow: out = sel^T rhs_hi + sel^T rhs_lo."""
                nc.tensor.matmul(
                    out_ap, sel_ap(sel_tile, ci), hilo_ap(rhs_tile, k, rs),
                    start=True, stop=True,
                    perf_mode=mybir.MatmulPerfMode.DoubleRow,
                )

            # --- global pipelined unit stream --------------------------
            # Flat list of pair-units spanning both layers and all r-tiles
            # so the bcast lookahead never drains at boundaries. Each unit:
            # optional pre() (PE corr matmuls opening an acc group), 2
            # bcast chunks into one 2-bank psum tile, a square (ACT) or
            # pair-multiply (DVE), 2 main matmuls, optional post().

            class Unit:
                __slots__ = ("rs", "rt", "chunks", "acc", "mech", "start",
                             "stop", "pre", "post", "gate")

                def __init__(self, rs, rt, chunks, acc, mech, start, stop,
                             pre=None, post=None, gate=-1):
                    self.rs, self.rt, self.chunks, self.acc = rs, rt, chunks, acc
                    self.mech = mech
                    self.start, self.stop = start, stop
                    self.pre, self.post = pre, post
                    # stream index whose post() must be emitted before this
                    # unit's bcast may read its rhs (V quantize ordering)
                    self.gate = gate

            accs = {}

            def get_acc(key):
                if key not in accs:
                    pool = psum_a0 if key[0] == "a0" else psum_a1
                    accs[key] = pool.tile([128, RT], f32, tag="acc",
                                          name="acc")
                return accs[key]

            state = {}  # rt -> [x2, c2, t]

            def make_x2(rt):
                rs = slice(rt * RT, (rt + 1) * RT)
                x2 = x2_pool.tile([32, RT], f16)
                nc.gpsimd.tensor_mul(x2, xt_sb[:, rs], xt_sb[:, rs])
                state[rt] = [x2, None, None]

            def l0_pre(rt):
                # x^2 for rt 0 and 1 primed up front; later rts prime from
                # l0_post (after the quantize ops, so Pool runs quantize first)
                def fn():
                    if rt == 0:
                        make_x2(0)
                        make_x2(1)
                return fn

            def l0_post(rt, rs):
                def fn():
                    acc0 = accs.pop(("a0", rt))
                    nc.tensor.matmul(acc0, wc0, state[rt][0],
                                     start=False, stop=True)
                    t = t_pool.tile([128, RT], bf16)
                    nc.scalar.activation(t, acc0, mybir.ActivationFunctionType.Relu)
                    state[rt][2] = t
                    # quantize cur0 into V tiles: hi = fp8(t), lo = t - hi
                    # (Pool). HW: engine APs from base partition 32 span <=32
                    # partitions, so the 96-row V1 write splits at 64.
                    lrs = slice(R + rs.start, R + rs.stop)
                    for pa, pb in ((32, 64), (64, 128)):
                        nc.gpsimd.tensor_copy(V1[pa:pb, rs], t[pa:pb, :])
                        nc.gpsimd.tensor_sub(V1[pa:pb, lrs], t[pa:pb, :], V1[pa:pb, rs])
                    nc.gpsimd.tensor_copy(V2[0:32, rs], t[0:32, :])
                    nc.gpsimd.tensor_sub(V2[0:32, lrs], t[0:32, :], V2[0:32, rs])
                    c2 = c2_pool.tile([128, RT], f16)
                    nc.gpsimd.tensor_mul(c2, t, t)
                    state[rt][1] = c2
                    if rt + 2 < NRT:
                        make_x2(rt + 2)
                    # reduce over d -> y0 (rows rotated; host undoes)
                    with nc.allow_low_precision("fp16 d-sum of 32 relu terms"):
                        nc.vector.tensor_reduce(
                            y0[:, rt * (RT // D):(rt + 1) * (RT // D)],
                            t.rearrange("p (b d) -> p b d", d=D),
                            axis=mybir.AxisListType.X,
                            op=mybir.AluOpType.add,
                        )
                return fn

            def l1_pre(rt):
                def fn():
                    v = SPLITPAT[rt % len(SPLITPAT)]
                    acc1 = get_acc(("a1", rt))
                    x2, c2, _t = state[rt]
                    nc.tensor.matmul(acc1, vt[v]["a1"], x2, start=True, stop=False)
                    nc.tensor.matmul(acc1, vt[v]["b1"], c2, start=False, stop=False)
                return fn

            def l1_post(rt):
                def fn():
                    del state[rt]
                    acc1 = accs.pop(("a1", rt))
                    u_t = relu1_pool.tile([128, RT], bf16)
                    nc.scalar.activation(u_t, acc1, mybir.ActivationFunctionType.Relu)
                    with nc.allow_low_precision("fp16 d-sum of 32 relu terms"):
                        nc.vector.tensor_reduce(
                            y1[:, rt * (RT // D):(rt + 1) * (RT // D)],
                            u_t.rearrange("p (b d) -> p b d", d=D),
                            axis=mybir.AxisListType.X,
                            op=mybir.AluOpType.add,
                        )
                    # stream completed output columns out early
                    if (rt + 1) % (NRT // 4) == 0:
                        q = (rt + 1) // (NRT // 4) - 1
                        cs = slice(q * (BC // 4), (q + 1) * (BC // 4))
                        nc.sync.dma_start(out=y[0][:, cs], in_=y0[:, cs])
                        nc.gpsimd.dma_start(out=y[1][:, cs], in_=y1[:, cs])
                return fn

            def l1_units(rt, gate):
                rs = slice(rt * RT, (rt + 1) * RT)
                v = SPLITPAT[rt % len(SPLITPAT)]
                na = v * 96 // 128
                # square-trick chunks: (sel tile, idx, rhs tile, K, w, wcol)
                sq = []
                for j in range(v):
                    if j < na:
                        sq.append((vt[v]["sel1a"], j, V1, 128, vt[v]["w1p"], j))
                    else:
                        sq.append((vt[v]["sel1b"], j - na, V2, 64, vt[v]["w1p"], j))
                dr = [(vt[v]["sel1d"], c, V1, 32, vt[v]["w1d"], c)
                      for c in range(F - v)]
                units = []
                nsq, ndr = len(sq) // 2, len(dr) // 2
                pat = os.environ.get("CIN_PAT", "ds")
                order = []
                if pat == "block":
                    for u in range(ndr):
                        order.append(("dir", dr[2 * u:2 * u + 2]))
                    for u in range(nsq):
                        order.append(("sq", sq[2 * u:2 * u + 2]))
                elif pat == "dds":
                    di = si = 0
                    while di < ndr or si < nsq:
                        for _ in range(2):
                            if di < ndr:
                                order.append(("dir", dr[2 * di:2 * di + 2])); di += 1
                        for _ in range(2):
                            if si < nsq:
                                order.append(("sq", sq[2 * si:2 * si + 2])); si += 1
                else:
                    for u in range(max(nsq, ndr)):       # interleave dir/sq
                        if u < ndr:
                            order.append(("dir", dr[2 * u:2 * u + 2]))
                        if u < nsq:
                            order.append(("sq", sq[2 * u:2 * u + 2]))
                for k, (mech, chunks) in enumerate(order):
                    units.append(Unit(
                        rs, rt, chunks, ("a1", rt), mech,
                        start=False, stop=(k == len(order) - 1),
                        pre=l1_pre(rt) if k == 0 else None,
                        post=l1_post(rt) if k == len(order) - 1 else None,
                        # direct bcasts read only x rows; square bcasts need
                        # the V quantize from this rt's l0_post
                        gate=gate if mech == "sq" else -1,
                    ))
                return units

            def build_stream():
                stream = []
                post_idx = {}
                for rt in range(NRT):
                    rs = slice(rt * RT, (rt + 1) * RT)
                    for u in (0, 1):
                        stream.append(Unit(
                            rs, rt,
                            [(sel0, 2 * u + hh, V1, 32, w0p, 2 * u + hh)
                             for hh in (0, 1)],
                            ("a0", rt), "sq",
                            start=(u == 0), stop=False,
                            pre=l0_pre(rt) if u == 0 else None,
                            post=l0_post(rt, rs) if u == 1 else None,
                        ))
                    post_idx[rt] = len(stream) - 1
                    if rt >= LAG:
                        stream.extend(l1_units(rt - LAG, post_idx[rt - LAG]))
                for rt in range(NRT - LAG, NRT):
                    stream.extend(l1_units(rt, post_idx[rt]))
                return stream

            def pump(stream):
                bc_q = {}
                next_e = 0

                def emit_ready(done, limit):
                    # emit pending bcasts up to `limit` whose gate post has
                    # been emitted (gate <= done)
                    nonlocal next_e
                    while (next_e < len(stream) and next_e <= limit
                           and stream[next_e].gate <= done):
                        un = stream[next_e]
                        s2 = psum_bc.tile([128, 2 * RT], f32, tag="s2")
                        for half, ch in enumerate(un.chunks):
                            st, ci, rtile, k = ch[:4]
                            dr_sum(st, ci, rtile, k, un.rs,
                                   s2[:, half * RT:(half + 1) * RT])
                        bc_q[next_e] = s2
                        next_e += 1

                emit_ready(-1, LOOKAHEAD)
                for i, un in enumerate(stream):
                    emit_ready(i - 1, i)  # self, if gated until now
                    if un.pre is not None:
                        un.pre()
                    s2 = bc_q.pop(i)
                    zt2 = zt_pool.tile([128, 2 * RT], f16)
                    if un.mech == "sq":
                        nc.scalar.activation(
                            zt2, s2, mybir.ActivationFunctionType.Square)
                    else:
                        t = state[un.rt][2]
                        nc.vector.tensor_mul(
                            zt2.rearrange("p (j f) -> p j f", j=2),
                            pair_ap(t),
                            s2.rearrange("p (j f) -> p j f", j=2),
                        )
                    emit_ready(i - 1, i + LOOKAHEAD + 1)
                    acc = get_acc(un.acc)
                    for half, ch in enumerate(un.chunks):
                        w_tile, wcol = ch[4], ch[5]
                        nc.tensor.matmul(
                            acc,
                            w_tile[:, wcol * 128:(wcol + 1) * 128],
                            zt2[:, half * RT:(half + 1) * RT],
                            start=(un.start and half == 0),
                            stop=(un.stop and half == 1),
                        )
                    if un.post is not None:
                        un.post()
                        emit_ready(i, i + LOOKAHEAD + 1)

            for _rep in range(REPEAT):
                pump(build_stream())


    nc.finalize()
    return nc


_NC_CACHE: bass.Bass | None = None


def _get_nc() -> bass.Bass:
    global _NC_CACHE
    if _NC_CACHE is None:
        _NC_CACHE = build_nc()
    return _NC_CACHE


def rebuild(repeat: int = 1) -> None:
    global _NC_CACHE, REPEAT
    REPEAT = repeat
    _NC_CACHE = None


def _pack_weights(w0: np.ndarray, w1: np.ndarray) -> dict[str, np.ndarray]:
    """w0 [F*F, 128], w1 [F*128, 128] fp32 -> device weight layouts."""
    W0 = w0.reshape(F, F, 128)
    G = W0 + W0.transpose(1, 0, 2)
    rows0 = 0.5 * np.stack([G[h, m] for (h, m) in PAIRS0])        # [496,128]
    rows0 = np.concatenate([rows0, np.zeros((L0C * 128 - len(rows0), 128))])
    corr0 = np.stack(
        [W0[h, h] - 0.5 * (G[h].sum(0) - G[h, h]) for h in range(F)])
    # rotate layer-0 output channels by ROT so t rows align with V layout
    rows0 = np.roll(rows0, ROT, axis=1)
    corr0 = np.roll(corr0, ROT, axis=1)
    w0p = np.ascontiguousarray(
        rows0.reshape(L0C, 128, 128).transpose(1, 0, 2)).reshape(128, -1)

    W1 = w1.reshape(F, 128, 128)
    nv = len(VARIANTS)
    out = {"w0p": w0p.astype(np.float16)}
    # wcb: per-variant b1 blocks, shared corr0, per-variant a1 blocks
    wcb = np.zeros((128, 128 * (2 * nv + 1)))
    wcb[0:32, 128 * nv:128 * (nv + 1)] = corr0
    for vi, v in enumerate(VARIANTS):
        h_dir, blk1, blk2 = _l1_blks(v)
        rows1 = 0.5 * np.stack([W1[h, m] for (h, m) in blk1 + blk2])
        out[f"w1p{v}"] = np.ascontiguousarray(
            rows1.reshape(v, 128, 128).transpose(1, 0, 2)
        ).reshape(128, -1).astype(np.float16)
        # direct chunks: row p multiplies zt[p] = x_h * t[p] (t rotated)
        rows1d = np.stack([np.roll(W1[h], ROT, axis=0) for h in h_dir])
        out[f"w1d{v}"] = np.ascontiguousarray(
            rows1d.transpose(1, 0, 2)).reshape(128, -1).astype(np.float16)
        # corrections: only square-trick (h, m) pairs contribute
        a1 = np.zeros((32, 128))
        for h in range(v):
            a1[h] = -0.5 * W1[h].sum(axis=0)
        b1 = -0.5 * W1[:v].sum(axis=0)     # [128 m, 128 n]
        wcb[:, 128 * vi:128 * (vi + 1)] = np.roll(b1, ROT, axis=0)
        wcb[0:32, 128 * (nv + 1 + vi):128 * (nv + 2 + vi)] = a1
    out["wcb"] = wcb.astype(np.float16)
    return out


def run(inputs, filter_0, filter_1, **spmd_kwargs):
    """Run on 8 NeuronCores; returns (out [4096, 256] f32, BassKernelResults)."""
    inputs = np.asarray(inputs, dtype=np.float32)
    w0 = np.asarray(filter_0, dtype=np.float32)[0]
    w1 = np.asarray(filter_1, dtype=np.float32)[0]
    assert inputs.shape == (B, F, D), inputs.shape
    assert w0.shape == (F * F, N0), w0.shape
    assert w1.shape == (F * N0, N1), w1.shape
    wmap = _pack_weights(w0, w1)

    nc = _get_nc()
    in_maps = []
    for i in range(NCORES):
        shard = inputs[i * BC:(i + 1) * BC]                       # [BC, F, D]
        xt = np.ascontiguousarray(shard.transpose(1, 0, 2)).reshape(F, R)
        x_hi = xt.astype(np_f8)
        x_lo = (xt - x_hi.astype(np.float32)).astype(np_f8)
        x8 = np.concatenate([x_hi, x_lo], axis=1)                 # [F, 2R]
        in_maps.append({"x8": x8, "xt": xt.astype(np_bf16), **wmap})
    res = run_bass_kernel_spmd(nc, in_maps, core_ids=list(range(NCORES)), **spmd_kwargs)
    parts = []
    for i in range(NCORES):
        yc = np.asarray(res.results[i]["y"]).astype(np.float32)
        y0n = np.roll(yc[0], -ROT, axis=0)  # undo layer-0 channel rotation
        parts.append(np.concatenate([y0n.T, yc[1].T], axis=1))    # [BC, 256]
    out = np.concatenate(parts, axis=0).astype(np.float32)        # [B, 256]
    return out, res


def kernel(inputs, filter_0, filter_1):
    out, _ = run(inputs, filter_0, filter_1)
    return out


if __name__ == "__main__":
    rng = np.random.default_rng(0)
    xs = rng.standard_normal((B, F, D)).astype(np.float32)
    f0 = (rng.standard_normal((1, F * F, N0)) * 0.05).astype(np.float32)
    f1 = (rng.standard_normal((1, F * N0, N1)) * 0.05).astype(np.float32)
    out = kernel(inputs=xs, filter_0=f0, filter_1=f1)
    print("kernel ran, out shape", out.shape, "mean", out.mean())
